# revision 6
# baseline (speedup 1.0000x reference)
"""Trainium2 Bass kernel for a per-token fake-quantized Linear:

    y = fake_quant(fake_quant(x) @ W.T + b)      (per-token int8 symmetric)

x: [4, 2048, 4096] f32, W: [4096, 4096] f32, b: [4096] f32.

Strategy (8 NeuronCores, pure data parallel over tokens - zero collectives):
  - 8192 tokens / 8 cores = 1024 tokens per core; W, b replicated.
  - Per-token quantized x values are integers in [-127, 127], EXACTLY
    representable in bf16, so the matmul runs on TensorE in bf16 with f32
    PSUM accumulation. The only precision loss vs the f32 reference is W's
    bf16 rounding (~0.8% rel err after output requant; gate is 2e-2).
  - Rounding is exact round-to-nearest-even via +/-1.5*2^23 magic adds.
  - NATURAL-LAYOUT output: the matmul uses q^T strips as the STATIONARY
    operand (sliced per 128-token tile) and the host-packed W^T blocks as
    the MOVING operand (w[:, s, :] = [128k, 512o]), so PSUM holds
    y[128 tokens, 512 outs] directly. No output transpose, no DRAM
    round-trip: per-token absmax accumulates column-by-column during PSUM
    evacuation and requant reads y straight from SBUF.
  - Two phases of 4 token tiles each (t0-3, then t4-7); W is streamed from
    HBM once per phase (64 MiB total - the DMA queues have the headroom).
    Phase B's quant runs under phase A's matmuls; phase A's requant runs
    under phase B's matmuls; only phase B's requant (~20us) trails.
  - Bias is folded into the matmul as a K=1 rank-1 update
    (rinv_row_chunk^T @ b_row_chunk) since s_x * rinv_x == 1.
  - Lead-in: the 4 x tiles load as full [128, 4096] rows on 4 different
    DMA rings (sync/scalar/vector/tensor) so descriptor generation and
    queue traffic parallelize; W og0's blocks prefetch on the tensor ring
    before any matmul is queued behind them. q^T strip copies are emitted
    kb-group-major across tiles and split across scalar/vector/gpsimd so
    og0's first block (k0-7 strips, 32 copies) is ready ASAP.
"""

import sys

if "/opt/trn_rl_repo" not in sys.path:
    sys.path.insert(0, "/opt/trn_rl_repo")

from contextlib import ExitStack

import ml_dtypes
import numpy as np

import concourse.bass as bass
import concourse.mybir as mybir
import concourse.tile as tile
from concourse import bacc
from concourse.bass import ds
from concourse.bass_utils import run_bass_kernel_spmd
from concourse.masks import make_identity

N_CORES = 8
P = 128
T = 1024          # tokens per core
K = 4096          # in features
O = 4096          # out features
TT = T // P       # 8 token tiles
KT = K // P       # 32 k tiles
TH = T // 2       # tokens per phase (512)
NTP = TH // P     # token tiles per phase (4)
OG = 512          # outputs per o-group (one PSUM bank per token tile)
NOG = O // OG     # 8 o-groups
OQ = O // 8       # requant chunk

Q_MAX = 127.0
EPS = 1e-5
MAGIC = 1.5 * 2**23  # f32 add/sub forces round-to-nearest-even to integer
INV_QMAX = float(np.float32(1.0) / np.float32(Q_MAX))

F32 = mybir.dt.float32
BF16 = mybir.dt.bfloat16


KB = 8                       # k-subtiles per W block
NKB = KT // KB               # 4 blocks per o-group
NBLK = NKB * NOG             # 32 1-MiB W blocks
KH = K // 2


def build():
    nc = bacc.Bacc()
    x_ext = nc.declare_dram_parameter("x", [T, K], F32, isOutput=False)
    # W is HOST-PACKED per (kb, og) block so each block is a fully
    # contiguous [128, 4096] read (128 descriptors of 8 KiB).
    # block(kb, og)[p, s, o'] = W^T[kb*1024 + s*128 + p, og*512 + o'].
    wt_ext = nc.declare_dram_parameter("wt", [NBLK, P, KB * OG], BF16,
                                       isOutput=False)
    b_ext = nc.declare_dram_parameter("b", [O], F32, isOutput=False)
    out_ext = nc.declare_dram_parameter("out", [T, O], F32, isOutput=True)

    with tile.TileContext(nc) as tc, ExitStack() as ctx:
        dram = ctx.enter_context(tc.tile_pool(name="dram", bufs=1, space="DRAM"))
        singles = ctx.enter_context(tc.tile_pool(name="singles", bufs=1))
        xp = ctx.enter_context(tc.tile_pool(name="xp", bufs=2))      # 32K/part
        wp = ctx.enter_context(tc.tile_pool(name="wp", bufs=3))      # 24K/part
        qp = ctx.enter_context(tc.tile_pool(name="qp", bufs=2))      # 8K/part
        qt_pool = ctx.enter_context(tc.tile_pool(name="qt", bufs=1))  # 64K/part
        ysb_pool = ctx.enter_context(tc.tile_pool(name="ysb", bufs=1))  # 64K/part
        sxp = ctx.enter_context(tc.tile_pool(name="sxp", bufs=1))
        stat = ctx.enter_context(tc.tile_pool(name="stat", bufs=4))
        yp = ctx.enter_context(tc.tile_pool(name="yp", bufs=2))      # 8K/part
        psum = ctx.enter_context(tc.tile_pool(name="psum", bufs=5, space="PSUM"))
        tpp = ctx.enter_context(tc.tile_pool(name="tpp", bufs=2, space="PSUM"))

        rinv_dram = dram.tile([TT, P], F32, tag="rinv_dram")

        identity = singles.tile([P, P], BF16, tag="identity")
        make_identity(nc, identity)

        # bias row in bf16 (partition 0): moving operand of the K=1 bias
        # matmul (stationary = rinv chunk, so psum[t, o] += rinv[t]*b[o])
        b_row = singles.tile([1, O], BF16, tag="b_row")
        nc.gpsimd.dma_start(out=b_row, in_=b_ext[:])  # gpsimd DMA casts f32->bf16

        # q^T strips, one per (phase, k-tile): [128k, 512t] bf16.
        # matmul stationary = qt_tiles[h][k][:, t*128:(t+1)*128]
        qt_tiles = [
            [qt_pool.tile([P, TH], BF16, tag=f"qt{h}_{k}", name=f"qt{h}_{k}")
             for k in range(KT)]
            for h in range(2)
        ]
        # y rows (q units) accumulated per token tile across og evacs
        ysb_tiles = [
            ysb_pool.tile([P, O], BF16, tag=f"ysb{t}", name=f"ysb{t}")
            for t in range(TT)
        ]
        # per-token running |y| max, one column per og
        amz_tiles = [
            stat.tile([P, NOG], F32, tag=f"amz{t}", bufs=1, name=f"amz{t}")
            for t in range(TT)
        ]

        sx_tiles = [None] * TT
        rinv_rows = [None, None]

        def load_x(t, eng):
            """Issue tile t's full-row x load on the given DMA ring."""
            x_t = xp.tile([P, K], F32, tag="xp", name=f"x_{t}")
            eng.dma_start(out=x_t, in_=x_ext[ds(t * P, P), :])
            return x_t

        def quant_tile(t, x_t):
            """Quantize token tile t (x already loading/loaded); emit the
            PE transposes but NOT the strip copies (batched separately)."""
            am = stat.tile([P, 1], F32, tag="am_c")
            nc.vector.tensor_reduce(
                out=am, in_=x_t, axis=mybir.AxisListType.X,
                op=mybir.AluOpType.max, apply_absolute_value=True,
            )
            sx = sxp.tile([P, 1], F32, tag=f"sx{t}", name=f"sx{t}")
            # s = max(absmax, EPS) * (1/127)
            nc.vector.tensor_scalar(
                out=sx, in0=am, scalar1=EPS, scalar2=INV_QMAX,
                op0=mybir.AluOpType.max, op1=mybir.AluOpType.mult,
            )
            rinv = stat.tile([P, 1], F32, tag="rinv_x")
            nc.vector.reciprocal(out=rinv, in_=sx)
            nc.gpsimd.dma_start(out=rinv_dram[t, :], in_=rinv[:, 0:1])
            tps = []
            for i in range(2):
                # r = x * rinv + MAGIC  (in place, gpsimd), q = r - MAGIC -> bf16
                nc.gpsimd.tensor_scalar(
                    out=x_t[:, ds(i * KH, KH)], in0=x_t[:, ds(i * KH, KH)],
                    scalar1=rinv, scalar2=MAGIC,
                    op0=mybir.AluOpType.mult, op1=mybir.AluOpType.add,
                )
                q_half = qp.tile([P, KH], BF16, tag="q_half")
                nc.vector.tensor_scalar(
                    out=q_half, in0=x_t[:, ds(i * KH, KH)], scalar1=MAGIC,
                    scalar2=None, op0=mybir.AluOpType.subtract,
                )
                for j in range(KT // 2):
                    tp = tpp.tile([P, P], BF16, tag="tp")
                    nc.tensor.transpose(tp, q_half[:, ds(j * P, P)], identity)
                    tps.append(tp)
            sx_tiles[t] = sx
            return tps

        def copy_strip(h, k, t, tp, eng):
            row = (t % NTP) * P
            if eng is nc.scalar:
                eng.copy(out=qt_tiles[h][k][:, ds(row, P)], in_=tp)
            else:
                eng.tensor_copy(out=qt_tiles[h][k][:, ds(row, P)], in_=tp)

        def load_rinv_row(h):
            # rinv as a bf16 row [1, TH]: stationary chunks of the K=1
            # bias matmul (free dim = tokens)
            r = singles.tile([1, TH], BF16, tag=f"rinv_row{h}",
                             name=f"rinv_row{h}")
            nc.gpsimd.dma_start(out=r, in_=rinv_dram[ds(h * NTP, NTP), :])
            rinv_rows[h] = r

        def matmul_og(h, og, w_engine_of, evac_eng):
            ts = [h * NTP + i for i in range(NTP)]
            ps = [
                psum.tile([P, OG], F32, tag="ps", name=f"ps_{h}_{og}_{i}")
                for i in range(NTP)
            ]
            for kb in range(NKB):
                w_tile = wp.tile([P, KB, OG], BF16, tag="wp", name="w_tile")
                w_engine_of(kb).dma_start(
                    out=w_tile,
                    in_=wt_ext[kb * NOG + og].rearrange("p (s o) -> p s o",
                                                        o=OG),
                )
                for s in range(KB):
                    k = kb * KB + s
                    for i in range(NTP):
                        nc.tensor.matmul(
                            ps[i],
                            qt_tiles[h][k][:, ds(i * P, P)],
                            w_tile[:, s, :],
                            start=(k == 0),
                            stop=False,
                        )
            # bias: psum[t, o] += rinv[t] * b[o]   (K=1 matmul)
            for i in range(NTP):
                nc.tensor.matmul(
                    ps[i],
                    rinv_rows[h][0:1, ds(i * P, P)],
                    b_row[0:1, ds(og * OG, OG)],
                    start=False,
                    stop=True,
                )
            for i in range(NTP):
                t = ts[i]
                evac_eng.tensor_copy(
                    out=ysb_tiles[t][:, ds(og * OG, OG)], in_=ps[i]
                )
                nc.vector.tensor_reduce(
                    out=amz_tiles[t][:, og:og + 1], in_=ps[i],
                    axis=mybir.AxisListType.X,
                    op=mybir.AluOpType.max, apply_absolute_value=True,
                )

        def requant_tile(t, store_eng):
            """Requantize token tile t from SBUF y rows and store."""
            am = stat.tile([P, 1], F32, tag="am_z")
            nc.vector.tensor_reduce(
                out=am, in_=amz_tiles[t], axis=mybir.AxisListType.X,
                op=mybir.AluOpType.max,
            )
            sy = stat.tile([P, 1], F32, tag="sy")
            # sy = (max(am * sx, EPS)) * (1/127)
            nc.vector.tensor_scalar(
                out=sy, in0=am, scalar1=sx_tiles[t], scalar2=EPS,
                op0=mybir.AluOpType.mult, op1=mybir.AluOpType.max,
            )
            nc.vector.tensor_scalar(
                out=sy, in0=sy, scalar1=INV_QMAX, scalar2=None,
                op0=mybir.AluOpType.mult,
            )
            rinvy = stat.tile([P, 1], F32, tag="rinv_y")
            nc.vector.reciprocal(out=rinvy, in_=sy)
            # f1 = s_x * rinv_y: ONE scalar-engine activation then does
            # r = z*f1 + MAGIC (bias already inside z)
            f1 = stat.tile([P, 1], F32, tag="f1")
            nc.vector.tensor_scalar(
                out=f1, in0=rinvy, scalar1=sx_tiles[t], scalar2=None,
                op0=mybir.AluOpType.mult,
            )
            for i in range(O // OQ):
                y_q = yp.tile([P, OQ], F32, tag="y_q")
                # r = z * (sx*rinvy) + MAGIC  (scalar ACT, fused affine)
                nc.scalar.activation(
                    out=y_q, in_=ysb_tiles[t][:, ds(i * OQ, OQ)],
                    func=mybir.ActivationFunctionType.Copy,
                    bias=MAGIC, scale=f1,
                )
                # y_q = (r - MAGIC) * s_y  (in place; vector ONLY - gpsimd
                # tensor_scalar with an AP in the scalar2 slot is slow)
                nc.vector.tensor_scalar(
                    out=y_q, in0=y_q, scalar1=MAGIC, scalar2=sy,
                    op0=mybir.AluOpType.subtract, op1=mybir.AluOpType.mult,
                )
                store_eng.dma_start(
                    out=out_ext[ds(t * P, P), ds(i * OQ, OQ)], in_=y_q
                )

        # ---- lead-in: x tiles 0-3 split across the two HWDGE rings; W og0
        # queues on sync BEHIND x (FIFO gives x priority; og0-kb0 lands
        # before the first matmul needs it) ----
        x_rings = [nc.sync, nc.scalar, nc.sync, nc.scalar]
        x_tiles = [load_x(t, x_rings[t]) for t in range(NTP)]
        # quant chains (vector reduce -> gpsimd magic -> vector cast -> PE
        # transposes); strip copies batched kb-major across tiles below
        tps_all = [quant_tile(t, x_tiles[t]) for t in range(NTP)]
        load_rinv_row(0)
        # kb-group-major strip copies split across scalar/gpsimd/vector so
        # og0 (k0-7, all 4 tiles) completes first
        copy_engs = [nc.scalar, nc.gpsimd, nc.vector]
        ci = 0
        for kb in range(NKB):
            for t in range(NTP):
                for s in range(KB):
                    k = kb * KB + s
                    copy_strip(0, k, t, tps_all[t][k], copy_engs[ci % 3])
                    ci += 1

        # ---- phase A: og sweep for token tiles 0-3 ----
        # W: og0 prefetched on tensor (before matmuls queue), rest on sync.
        xb_tiles = [None] * NTP
        tps_b = [None] * NTP
        for og in range(NOG):
            matmul_og(0, og, w_engine_of=lambda kb: nc.sync,
                      evac_eng=nc.vector)
            if 1 <= og <= 4:
                # phase B quant, one tile per og boundary; x on scalar ring
                i = og - 1
                xb_tiles[i] = load_x(NTP + i, nc.scalar)
                tps_b[i] = quant_tile(NTP + i, xb_tiles[i])
                # strip copies for this tile (scalar + gpsimd; vector is
                # busy with evacs)
                for k in range(KT):
                    copy_strip(1, k, NTP + i, tps_b[i][k],
                               copy_engs[ci % 2])
                    ci += 1
                if og == 4:
                    load_rinv_row(1)

        # ---- phase B: og sweep for token tiles 4-7; phase A requant
        # interleaved ----
        for og in range(NOG):
            matmul_og(1, og, w_engine_of=lambda kb: nc.sync,
                      evac_eng=nc.vector)
            if 1 <= og <= 4:
                requant_tile(og - 1, store_eng=nc.gpsimd)

        # ---- tail: requant token tiles 4-7 ----
        for t in range(NTP, TT):
            requant_tile(t, store_eng=nc.gpsimd)

    nc.compile()
    return nc


_NC_CACHE = None


def _get_nc():
    global _NC_CACHE
    if _NC_CACHE is None:
        _NC_CACHE = build()
    return _NC_CACHE


def _run(x, W, b, trace=False):
    nc = _get_nc()
    x2d = np.ascontiguousarray(np.asarray(x, dtype=np.float32).reshape(-1, K))
    wt = np.asarray(W, dtype=np.float32).T.astype(ml_dtypes.bfloat16)
    # pack into per-(kb, og) contiguous blocks: [NBLK, 128, KB*OG]
    wt = np.ascontiguousarray(
        wt.reshape(NKB, KB, P, NOG, OG)
        .transpose(0, 3, 2, 1, 4)
        .reshape(NBLK, P, KB * OG)
    )
    bf = np.ascontiguousarray(np.asarray(b, dtype=np.float32))
    in_maps = [
        {"x": np.ascontiguousarray(x2d[i * T:(i + 1) * T]), "wt": wt, "b": bf}
        for i in range(N_CORES)
    ]
    res = run_bass_kernel_spmd(nc, in_maps, list(range(N_CORES)), trace=trace)
    out = np.concatenate([res.results[i]["out"] for i in range(N_CORES)], axis=0)
    return out, res


def kernel(x, W, b):
    out, _ = _run(x, W, b, trace=False)
    return out.reshape(np.asarray(x).shape[:-1] + (O,)).astype(np.float32)
